# revision 14
# baseline (speedup 1.0000x reference)
"""Multi-head attention (B=4, N=2048, C=1024, H=16, D=64) on 8 TRN2 cores.

Sharding: core c -> batch b = c%4, head-group g = c//4 (local heads 0..7 are
global heads 8g..8g+7).  Each core computes its head group's contribution to
the output projection for its batch; host sums core b + core b+4 and adds
const_row = qkv_b[2048:] @ proj_w + proj_b (V-bias folds exactly through the
row-normalized attention: attn @ (1*bv^T) = 1*bv^T).

v3 structure (per core):
  phase 1: K and V projections only (Q is interleaved into phase 2 so its
    matmuls fill Tensor-engine gaps while Scalar/Vector run exp).  Startup
    DMAs are section-ordered so the first K matmul can start early.
  phase 2: per query block (512 q) x head pair: row-packed K=64 score
    matmuls -> PSUM [128 keys, 1024]; exp split between Scalar (exact, 2/3)
    and Vector (Schraudolph bf16 bit-trick via tensor_scalar->uint16, 1/3);
    P bf16 -> PV matmuls accumulate oaug [65, 512] (col 64 of V = ones =>
    row 64 = softmax denominator).  oaug is immediately evacuated PSUM->SBUF
    (oc), denominator rows hop to den8 via SBUF->SBUF DMA, one batched exact
    reciprocal per q-block, DRAM-bounce broadcast, GpSimd tensor_tensor
    multiply -> O_qb bf16.  The projection of q-block i is DEFERRED and
    emitted during q-block i+1 so its PSUM ring never blocks the next
    block's PV matmuls.

Device layouts (per core):
  xT   [1024, 2048]  x[b].T (C x N), fp32r-prerounded
  wcat [1024, 1536]  [Wq | Wk | Wv] cols for this head group, fp32r
  qb/kb [128, 4]     bias chunk pr in column pr
  pw   [512, 1024]   proj_w rows for this head group's channels, bf16
  out  [2048, 1024]  partial projection output
  Q_T/K_T [128, 4, 2048] fp32r: partition = dim-in-pair (2 heads x 64)
  V_sb [128, 16, 8, 65] bf16: partition = token-in-block; col 64 = ones
"""

import sys

sys.path.insert(0, "/opt/trn_rl_repo")

from contextlib import ExitStack

import ml_dtypes
import numpy as np

from concourse import bacc, mybir, tile
from concourse.bass_utils import run_bass_kernel_spmd

F32 = mybir.dt.float32
F32R = mybir.dt.float32r
BF16 = mybir.dt.bfloat16
U16 = mybir.dt.uint16
EXP = mybir.ActivationFunctionType.Exp
ADD = mybir.AluOpType.add
MULT = mybir.AluOpType.mult

B, N, C, H, D = 4, 2048, 1024, 16, 64
SCALE = 0.125
LOG2E = 1.4426950408889634
SCHR_C1 = SCALE * 128.0 * LOG2E
SCHR_C2 = 16256.0 - 5.5098
# exp tile -> DVE (Schraudolph) when (2*tg + which) % DVE_MOD == DVE_MOD - 1
DVE_MOD = 3


def _round_fp32r(a: np.ndarray) -> np.ndarray:
    b = np.ascontiguousarray(a, dtype=np.float32).view(np.uint32).astype(np.uint64)
    lsb = (b >> np.uint64(12)) & np.uint64(1)
    b = (b + np.uint64(0x7FF) + lsb) & np.uint64(0xFFFFF000)
    return b.astype(np.uint32).view(np.float32)


def _build():
    nc = bacc.Bacc("TRN2", target_bir_lowering=False, debug=False)
    xT = nc.dram_tensor("xT", [1024, 2048], F32, kind="ExternalInput").ap()
    wcat = nc.dram_tensor("wcat", [1024, 1536], F32, kind="ExternalInput").ap()
    qb = nc.dram_tensor("qb", [128, 4], F32, kind="ExternalInput").ap()
    kb = nc.dram_tensor("kb", [128, 4], F32, kind="ExternalInput").ap()
    pw = nc.dram_tensor("pw", [512, 1024], BF16, kind="ExternalInput").ap()
    out = nc.dram_tensor("out", [2048, 1024], F32, kind="ExternalOutput").ap()
    scratch = nc.dram_tensor("scratch", [4, 8, 512], F32).ap()

    with tile.TileContext(nc) as tc, ExitStack() as ctx:
        sb = ctx.enter_context(tc.tile_pool(name="sb", bufs=1))
        ps = ctx.enter_context(tc.tile_pool(name="ps", bufs=1, space="PSUM"))

        w_sb = sb.tile([128, 8, 1536], F32R, tag="w")
        Q_T = sb.tile([128, 4, 2048], F32R, tag="qt")
        K_T = sb.tile([128, 4, 2048], F32R, tag="kt")
        V_sb = sb.tile([128, 16, 8, 65], BF16, tag="v")
        qb_sb = sb.tile([128, 4], F32, tag="qb")
        kb_sb = sb.tile([128, 4], F32, tag="kb")
        zc = sb.tile([128, 8, 1], F32, tag="zc")
        onec = sb.tile([128, 1], F32, tag="onec")
        pw_sb = sb.tile([128, 4, 1024], BF16, tag="pw")

        def load_slab(nb):
            slab = sb.tile([128, 8, 256], F32R, tag="xslab", bufs=2)
            for j in range(8):
                nc.sync.dma_start(
                    slab[:, j, :],
                    xT[j * 128:(j + 1) * 128,
                       nb * 256:(nb + 1) * 256].bitcast(F32R))
            return slab

        # startup: first slab + K-section weights first so K matmuls start asap
        slab0 = load_slab(0)
        for j in range(8):
            nc.sync.dma_start(w_sb[:, j, 512:1024],
                              wcat[j * 128:(j + 1) * 128, 512:1024].bitcast(F32R))
        nc.sync.dma_start(kb_sb[:], kb[:])
        for j in range(8):
            nc.sync.dma_start(w_sb[:, j, 1024:1536],
                              wcat[j * 128:(j + 1) * 128, 1024:1536].bitcast(F32R))
        for j in range(8):
            nc.sync.dma_start(w_sb[:, j, 0:512],
                              wcat[j * 128:(j + 1) * 128, 0:512].bitcast(F32R))
        nc.sync.dma_start(qb_sb[:], qb[:])
        for pr in range(4):
            nc.sync.dma_start(pw_sb[:, pr, :], pw[pr * 128:(pr + 1) * 128, :])
        nc.vector.memset(zc[:], 0.0)
        nc.vector.memset(onec[:], 1.0)
        for t in range(16):
            nc.gpsimd.tensor_scalar(out=V_sb[:, t, :, 64:65], in0=zc[:],
                                    scalar1=onec[:], scalar2=None, op0=ADD)

        # Phase 1: K and V projections, 8 token blocks of 256
        for nb in range(8):
            slab = slab0 if nb == 0 else load_slab(nb)
            for pr in range(4):
                acc = ps.tile([128, 512], F32, tag="stage", bufs=2)
                for j in range(8):
                    nc.tensor.matmul(acc[:, 0:256],
                                     w_sb[:, j, 512 + pr * 128:512 + (pr + 1) * 128],
                                     slab[:, j, :], start=(j == 0), stop=(j == 7))
                nc.vector.tensor_scalar(out=K_T[:, pr, nb * 256:(nb + 1) * 256],
                                        in0=acc[:, 0:256],
                                        scalar1=kb_sb[:, pr:pr + 1],
                                        scalar2=None, op0=ADD)
            for kt2 in range(2):
                t = nb * 2 + kt2
                acc = ps.tile([128, 512], F32, tag="stage", bufs=2)
                for j in range(8):
                    nc.tensor.matmul(acc[:],
                                     slab[:, j, kt2 * 128:(kt2 + 1) * 128],
                                     w_sb[:, j, 1024:1536],
                                     start=(j == 0), stop=(j == 7))
                nc.scalar.copy(out=V_sb[:, t, :, 0:64],
                               in_=acc[:].rearrange("p (h d) -> p h d", h=8))

        def emit_q(nb):
            slab = load_slab(nb)
            for pr in range(4):
                acc = ps.tile([128, 512], F32, tag="stage", bufs=2)
                for j in range(8):
                    nc.tensor.matmul(acc[:, 0:256],
                                     w_sb[:, j, pr * 128:(pr + 1) * 128],
                                     slab[:, j, :], start=(j == 0), stop=(j == 7))
                nc.vector.tensor_scalar(out=Q_T[:, pr, nb * 256:(nb + 1) * 256],
                                        in0=acc[:, 0:256],
                                        scalar1=qb_sb[:, pr:pr + 1],
                                        scalar2=None, op0=ADD)

        def emit_proj(qb_i, O_qb, half):
            q0 = qb_i * 512
            for ns in (half * 2, half * 2 + 1):
                for co in range(2):
                    pj = ps.tile([128, 512], F32, tag="acc", bufs=4)
                    for pr in range(4):
                        nc.tensor.matmul(pj[:],
                                         O_qb[:, pr, ns * 128:(ns + 1) * 128],
                                         pw_sb[:, pr, co * 512:(co + 1) * 512],
                                         start=(pr == 0), stop=(pr == 3))
                    so = sb.tile([128, 512], F32, tag="stout", bufs=3)
                    if co == 0:
                        nc.vector.tensor_copy(out=so[:], in_=pj[:])
                    else:
                        nc.scalar.copy(out=so[:], in_=pj[:])
                    nc.sync.dma_start(
                        out[q0 + ns * 128:q0 + (ns + 1) * 128,
                            co * 512:(co + 1) * 512], so[:])

        # Q for the first query block
        emit_q(0)
        emit_q(1)

        # Phase 2: attention; proj of block i deferred into block i+1
        pending = None
        for qb_i in range(4):
            q0 = qb_i * 512
            O_qb = sb.tile([128, 4, 512], BF16, tag="oqb", bufs=2)
            den8 = sb.tile([8, 512], F32, tag="den8", bufs=2)
            oaugs = []
            for pr in range(4):
                oaug0 = ps.tile([65, 512], F32, tag="acc", bufs=4)
                oaug1 = ps.tile([65, 512], F32, tag="acc", bufs=4)
                staged = []
                for tg in range(9):
                    if tg < 8:
                        t0, t1 = 2 * tg, 2 * tg + 1
                        stage0 = ps.tile([128, 1024], F32, tag="stage", bufs=2)
                        stage1 = ps.tile([128, 1024], F32, tag="stage", bufs=2)
                        nc.tensor.matmul(stage0[:, 0:512],
                                         K_T[0:64, pr, t0 * 128:(t0 + 1) * 128],
                                         Q_T[0:64, pr, q0:q0 + 512],
                                         start=True, stop=True, tile_position=(0, 0))
                        nc.tensor.matmul(stage1[:, 0:512],
                                         K_T[64:128, pr, t0 * 128:(t0 + 1) * 128],
                                         Q_T[64:128, pr, q0:q0 + 512],
                                         start=True, stop=True, tile_position=(64, 0))
                        nc.tensor.matmul(stage0[:, 512:1024],
                                         K_T[0:64, pr, t1 * 128:(t1 + 1) * 128],
                                         Q_T[0:64, pr, q0:q0 + 512],
                                         start=True, stop=True, tile_position=(0, 0))
                        nc.tensor.matmul(stage1[:, 512:1024],
                                         K_T[64:128, pr, t1 * 128:(t1 + 1) * 128],
                                         Q_T[64:128, pr, q0:q0 + 512],
                                         start=True, stop=True, tile_position=(64, 0))
                    if tg >= 1:
                        pP0, pP1, pt0, pt1 = staged[tg - 1]
                        st, sp = (tg - 1 == 0), (tg - 1 == 7)
                        nc.tensor.matmul(oaug0[:], V_sb[:, pt0, 2 * pr, :],
                                         pP0[:, 0:512], start=st, stop=False)
                        nc.tensor.matmul(oaug0[:], V_sb[:, pt1, 2 * pr, :],
                                         pP0[:, 512:1024], start=False, stop=sp)
                        nc.tensor.matmul(oaug1[:], V_sb[:, pt0, 2 * pr + 1, :],
                                         pP1[:, 0:512], start=st, stop=False)
                        nc.tensor.matmul(oaug1[:], V_sb[:, pt1, 2 * pr + 1, :],
                                         pP1[:, 512:1024], start=False, stop=sp)
                    if tg < 8:
                        P0 = sb.tile([128, 1024], BF16, tag="p", bufs=4)
                        P1 = sb.tile([128, 1024], BF16, tag="p", bufs=4)
                        for which, (st_t, P) in enumerate(((stage0, P0),
                                                           (stage1, P1))):
                            if (2 * tg + which) % DVE_MOD == DVE_MOD - 1:
                                nc.vector.tensor_scalar(
                                    out=P.bitcast(U16), in0=st_t[:],
                                    scalar1=SCHR_C1, scalar2=SCHR_C2,
                                    op0=MULT, op1=ADD)
                            else:
                                nc.scalar.activation(P[:], st_t[:], EXP,
                                                     bias=0.0, scale=SCALE)
                        staged.append((P0, P1, t0, t1))
                # evacuate oaug PSUM->SBUF; hop denominator row into den8
                for hh, oaug in ((0, oaug0), (1, oaug1)):
                    row = pr * 2 + hh
                    oc = sb.tile([65, 512], F32, tag="ocp", bufs=8)
                    nc.vector.tensor_copy(out=oc[:], in_=oaug[:])
                    nc.sync.dma_start(den8[row:row + 1, :], oc[64:65, :])
                    oaugs.append(oc)
                if pr == 1 or pr == 3:
                    half = 0 if pr == 1 else 1
                    if pending is not None:
                        emit_proj(pending[0], pending[1], half)
                    if qb_i < 3:
                        emit_q(2 * (qb_i + 1) + half)
            rec8 = sb.tile([8, 512], F32, tag="rec8", bufs=2)
            nc.vector.reciprocal(rec8[:], den8[:])
            nc.sync.dma_start(scratch[qb_i, :, :], rec8[:])
            for pr in range(4):
                for hh in (0, 1):
                    row = pr * 2 + hh
                    oc = oaugs[row]
                    rb = sb.tile([64, 512], F32, tag="rb", bufs=4)
                    nc.sync.dma_start(
                        rb[:],
                        scratch[qb_i, row:row + 1, :].to_broadcast((64, 512)))
                    nc.gpsimd.tensor_tensor(
                        out=O_qb[hh * 64:(hh + 1) * 64, pr, :],
                        in0=oc[0:64, :], in1=rb[:], op=MULT)
            pending = (qb_i, O_qb)
        emit_proj(pending[0], pending[1], 0)
        emit_proj(pending[0], pending[1], 1)
    return nc


def _to_bf16(a: np.ndarray) -> np.ndarray:
    return np.ascontiguousarray(a.astype(ml_dtypes.bfloat16))


def _prepare_in_maps(x, qkv_w, qkv_b, proj_w):
    xr = _round_fp32r(x)
    wr = _round_fp32r(qkv_w)
    qkv_b = np.asarray(qkv_b, dtype=np.float32)
    in_maps = []
    for c in range(8):
        b, g = c % 4, c // 4
        w0 = 512 * g
        in_maps.append({
            "xT": np.ascontiguousarray(xr[b].T),
            "wcat": np.ascontiguousarray(np.concatenate(
                [wr[:, w0:w0 + 512],
                 wr[:, 1024 + w0:1024 + w0 + 512],
                 wr[:, 2048 + w0:2048 + w0 + 512]], axis=1)),
            "qb": np.ascontiguousarray(qkv_b[w0:w0 + 512].reshape(4, 128).T),
            "kb": np.ascontiguousarray(
                qkv_b[1024 + w0:1024 + w0 + 512].reshape(4, 128).T),
            "pw": _to_bf16(proj_w[w0:w0 + 512, :]),
        })
    return in_maps


def _gather(parts, qkv_b, proj_w, proj_b):
    const_row = (np.asarray(qkv_b)[2048:].astype(np.float64)
                 @ np.asarray(proj_w).astype(np.float64)
                 + np.asarray(proj_b).astype(np.float64))
    out = np.empty((B, N, C), np.float32)
    for b in range(B):
        out[b] = (parts[b].astype(np.float64) + parts[b + 4].astype(np.float64)
                  + const_row).astype(np.float32)
    return out


def kernel(**inputs: np.ndarray) -> np.ndarray:
    x = np.asarray(inputs["x"], dtype=np.float32)
    qkv_w = np.asarray(inputs["qkv_w"], dtype=np.float32)
    qkv_b = np.asarray(inputs["qkv_b"], dtype=np.float32)
    proj_w = np.asarray(inputs["proj_w"], dtype=np.float32)
    proj_b = np.asarray(inputs["proj_b"], dtype=np.float32)

    in_maps = _prepare_in_maps(x, qkv_w, qkv_b, proj_w)
    nc = _build()
    nc.finalize()
    res = run_bass_kernel_spmd(nc, in_maps, list(range(8)))
    parts = [res.results[c]["out"] for c in range(8)]
    return _gather(parts, qkv_b, proj_w, proj_b)


if __name__ == "__main__":
    import tempfile
    import time

    from concourse.bass_utils import compile_bass_kernel

    t0 = time.time()
    nc = _build()
    nc.compile()
    with tempfile.TemporaryDirectory() as td:
        compile_bass_kernel(nc, td, neff_name="k.neff")
    print(f"COMPILE OK ({time.time() - t0:.0f}s)", flush=True)


# revision 16
# speedup vs baseline: 1.2217x; 1.2217x over previous
"""Multi-head attention (B=4, N=2048, C=1024, H=16, D=64) on 8 TRN2 cores.

Sharding: core c -> batch b = c%4, head-group g = c//4 (local heads 0..7 are
global heads 8g..8g+7).  Each core computes its head group's contribution to
the output projection for its batch; host sums core b + core b+4 and adds
const_row = qkv_b[2048:] @ proj_w + proj_b (V-bias folds exactly through the
row-normalized attention: attn @ (1*bv^T) = 1*bv^T).

v4 (v2c base + boundary deferral):
  - exp split across Scalar (exact, 3/4) and Vector (Schraudolph bf16
    bit-trick via tensor_scalar->uint16-bitcast, 1/4).
  - P, V, O, proj_w bf16; Q/K fp32r.
  - oaug [65,512] evacuated PSUM->SBUF right after PV (65-row copy costs
    the same as 1 row); denominator rows hop to den8 [8,512] via
    SBUF->SBUF DMA; ONE exact reciprocal per q-block; DRAM-bounce
    broadcast; GpSimd tensor_tensor normalize (SBUF-only, Pool engine
    otherwise idle).
  - the normalize+projection chunk of q-block i is emitted AFTER the
    first head-pair block of q-block i+1, so the pj copy-outs sitting in
    the in-order Scalar/Vector queues already have their inputs ready
    when the engines reach them (v2c stalled ~15us per q-block here; v3's
    mid-loop interleave starved ACT entirely).
  - startup DMAs section-ordered (K cols, then V, then Q) so the first K
    matmul starts ~4us in.
"""

import sys

sys.path.insert(0, "/opt/trn_rl_repo")

from contextlib import ExitStack

import ml_dtypes
import numpy as np

from concourse import bacc, mybir, tile
from concourse.bass_utils import run_bass_kernel_spmd

F32 = mybir.dt.float32
F32R = mybir.dt.float32r
BF16 = mybir.dt.bfloat16
U16 = mybir.dt.uint16
EXP = mybir.ActivationFunctionType.Exp
ADD = mybir.AluOpType.add
MULT = mybir.AluOpType.mult

B, N, C, H, D = 4, 2048, 1024, 16, 64
SCALE = 0.125
LOG2E = 1.4426950408889634
SCHR_C1 = SCALE * 128.0 * LOG2E
SCHR_C2 = 16256.0 - 5.5098
DVE_MOD = 4


def _round_fp32r(a: np.ndarray) -> np.ndarray:
    b = np.ascontiguousarray(a, dtype=np.float32).view(np.uint32).astype(np.uint64)
    lsb = (b >> np.uint64(12)) & np.uint64(1)
    b = (b + np.uint64(0x7FF) + lsb) & np.uint64(0xFFFFF000)
    return b.astype(np.uint32).view(np.float32)


def _build():
    nc = bacc.Bacc("TRN2", target_bir_lowering=False, debug=False)
    xT = nc.dram_tensor("xT", [1024, 2048], F32, kind="ExternalInput").ap()
    wcat = nc.dram_tensor("wcat", [1024, 1536], F32, kind="ExternalInput").ap()
    qb = nc.dram_tensor("qb", [128, 4], F32, kind="ExternalInput").ap()
    kb = nc.dram_tensor("kb", [128, 4], F32, kind="ExternalInput").ap()
    pw = nc.dram_tensor("pw", [512, 1024], BF16, kind="ExternalInput").ap()
    out = nc.dram_tensor("out", [2048, 1024], F32, kind="ExternalOutput").ap()
    scratch = nc.dram_tensor("scratch", [4, 8, 512], F32).ap()

    with tile.TileContext(nc) as tc, ExitStack() as ctx:
        sb = ctx.enter_context(tc.tile_pool(name="sb", bufs=1))
        ps = ctx.enter_context(tc.tile_pool(name="ps", bufs=1, space="PSUM"))

        w_sb = sb.tile([128, 8, 1536], F32R, tag="w")
        Q_T = sb.tile([128, 4, 2048], F32R, tag="qt")
        K_T = sb.tile([128, 4, 2048], F32R, tag="kt")
        V_sb = sb.tile([128, 16, 8, 65], BF16, tag="v")
        qb_sb = sb.tile([128, 4], F32, tag="qb")
        kb_sb = sb.tile([128, 4], F32, tag="kb")
        zc = sb.tile([128, 8, 1], F32, tag="zc")
        onec = sb.tile([128, 1], F32, tag="onec")
        pw_sb = sb.tile([128, 4, 1024], BF16, tag="pw")

        def load_slab(nb):
            slab = sb.tile([128, 8, 256], F32R, tag="xslab", bufs=2)
            for j in range(8):
                nc.sync.dma_start(
                    slab[:, j, :],
                    xT[j * 128:(j + 1) * 128,
                       nb * 256:(nb + 1) * 256].bitcast(F32R))
            return slab

        slab0 = load_slab(0)
        for j in range(8):
            nc.sync.dma_start(w_sb[:, j, 512:1024],
                              wcat[j * 128:(j + 1) * 128, 512:1024].bitcast(F32R))
        nc.sync.dma_start(kb_sb[:], kb[:])
        for j in range(8):
            nc.sync.dma_start(w_sb[:, j, 1024:1536],
                              wcat[j * 128:(j + 1) * 128, 1024:1536].bitcast(F32R))
        for j in range(8):
            nc.sync.dma_start(w_sb[:, j, 0:512],
                              wcat[j * 128:(j + 1) * 128, 0:512].bitcast(F32R))
        nc.sync.dma_start(qb_sb[:], qb[:])
        for pr in range(4):
            nc.sync.dma_start(pw_sb[:, pr, :], pw[pr * 128:(pr + 1) * 128, :])
        nc.vector.memset(zc[:], 0.0)
        nc.vector.memset(onec[:], 1.0)
        for t in range(16):
            nc.gpsimd.tensor_scalar(out=V_sb[:, t, :, 64:65], in0=zc[:],
                                    scalar1=onec[:], scalar2=None, op0=ADD)

        # Phase 1: QKV projection, 8 token blocks of 256 (K, V first; Q last)
        for nb in range(8):
            slab = slab0 if nb == 0 else load_slab(nb)
            for pr in range(4):
                acc = ps.tile([128, 512], F32, tag="stage", bufs=2)
                for j in range(8):
                    nc.tensor.matmul(acc[:, 0:256],
                                     w_sb[:, j, 512 + pr * 128:512 + (pr + 1) * 128],
                                     slab[:, j, :], start=(j == 0), stop=(j == 7))
                nc.vector.tensor_scalar(out=K_T[:, pr, nb * 256:(nb + 1) * 256],
                                        in0=acc[:, 0:256],
                                        scalar1=kb_sb[:, pr:pr + 1],
                                        scalar2=None, op0=ADD)
            for kt2 in range(2):
                t = nb * 2 + kt2
                acc = ps.tile([128, 512], F32, tag="stage", bufs=2)
                for j in range(8):
                    nc.tensor.matmul(acc[:],
                                     slab[:, j, kt2 * 128:(kt2 + 1) * 128],
                                     w_sb[:, j, 1024:1536],
                                     start=(j == 0), stop=(j == 7))
                nc.scalar.copy(out=V_sb[:, t, :, 0:64],
                               in_=acc[:].rearrange("p (h d) -> p h d", h=8))
            for pr in range(4):
                acc = ps.tile([128, 512], F32, tag="stage", bufs=2)
                for j in range(8):
                    nc.tensor.matmul(acc[:, 0:256],
                                     w_sb[:, j, pr * 128:(pr + 1) * 128],
                                     slab[:, j, :], start=(j == 0), stop=(j == 7))
                nc.vector.tensor_scalar(out=Q_T[:, pr, nb * 256:(nb + 1) * 256],
                                        in0=acc[:, 0:256],
                                        scalar1=qb_sb[:, pr:pr + 1],
                                        scalar2=None, op0=ADD)

        # Phase 2
        def attn_block(qb_i, pr, oaugs, den8):
            q0 = qb_i * 512
            oaug0 = ps.tile([65, 512], F32, tag="acc", bufs=4)
            oaug1 = ps.tile([65, 512], F32, tag="acc", bufs=4)
            staged = []
            for tg in range(9):
                if tg < 8:
                    t0, t1 = 2 * tg, 2 * tg + 1
                    stage0 = ps.tile([128, 1024], F32, tag="stage", bufs=2)
                    stage1 = ps.tile([128, 1024], F32, tag="stage", bufs=2)
                    nc.tensor.matmul(stage0[:, 0:512],
                                     K_T[0:64, pr, t0 * 128:(t0 + 1) * 128],
                                     Q_T[0:64, pr, q0:q0 + 512],
                                     start=True, stop=True, tile_position=(0, 0))
                    nc.tensor.matmul(stage1[:, 0:512],
                                     K_T[64:128, pr, t0 * 128:(t0 + 1) * 128],
                                     Q_T[64:128, pr, q0:q0 + 512],
                                     start=True, stop=True, tile_position=(64, 0))
                    nc.tensor.matmul(stage0[:, 512:1024],
                                     K_T[0:64, pr, t1 * 128:(t1 + 1) * 128],
                                     Q_T[0:64, pr, q0:q0 + 512],
                                     start=True, stop=True, tile_position=(0, 0))
                    nc.tensor.matmul(stage1[:, 512:1024],
                                     K_T[64:128, pr, t1 * 128:(t1 + 1) * 128],
                                     Q_T[64:128, pr, q0:q0 + 512],
                                     start=True, stop=True, tile_position=(64, 0))
                if tg >= 1:
                    pP0, pP1, pt0, pt1 = staged[tg - 1]
                    st, sp = (tg - 1 == 0), (tg - 1 == 7)
                    nc.tensor.matmul(oaug0[:], V_sb[:, pt0, 2 * pr, :],
                                     pP0[:, 0:512], start=st, stop=False)
                    nc.tensor.matmul(oaug0[:], V_sb[:, pt1, 2 * pr, :],
                                     pP0[:, 512:1024], start=False, stop=sp)
                    nc.tensor.matmul(oaug1[:], V_sb[:, pt0, 2 * pr + 1, :],
                                     pP1[:, 0:512], start=st, stop=False)
                    nc.tensor.matmul(oaug1[:], V_sb[:, pt1, 2 * pr + 1, :],
                                     pP1[:, 512:1024], start=False, stop=sp)
                if tg < 8:
                    P0 = sb.tile([128, 1024], BF16, tag="p", bufs=3)
                    P1 = sb.tile([128, 1024], BF16, tag="p", bufs=3)
                    for which, (st_t, P) in enumerate(((stage0, P0),
                                                       (stage1, P1))):
                        if (2 * tg + which) % DVE_MOD == DVE_MOD - 1:
                            nc.vector.tensor_scalar(
                                out=P.bitcast(U16), in0=st_t[:],
                                scalar1=SCHR_C1, scalar2=SCHR_C2,
                                op0=MULT, op1=ADD)
                        else:
                            nc.scalar.activation(P[:], st_t[:], EXP,
                                                 bias=0.0, scale=SCALE)
                    staged.append((P0, P1, t0, t1))
            for hh, oaug in ((0, oaug0), (1, oaug1)):
                row = pr * 2 + hh
                oc = sb.tile([65, 512], F32, tag="ocp", bufs=10)
                nc.vector.tensor_copy(out=oc[:], in_=oaug[:])
                nc.sync.dma_start(den8[row:row + 1, :], oc[64:65, :])
                oaugs.append(oc)

        def boundary_chunk(qb_i, O_qb, den8, oaugs):
            # normalize + projection of q-block qb_i
            q0 = qb_i * 512
            rec8 = sb.tile([8, 512], F32, tag="rec8", bufs=2)
            nc.vector.reciprocal(rec8[:], den8[:])
            nc.sync.dma_start(scratch[qb_i, :, :], rec8[:])
            for pr in range(4):
                for hh in (0, 1):
                    row = pr * 2 + hh
                    rb = sb.tile([64, 512], F32, tag="rb", bufs=3)
                    nc.sync.dma_start(
                        rb[:],
                        scratch[qb_i, row:row + 1, :].to_broadcast((64, 512)))
                    nc.gpsimd.tensor_tensor(
                        out=O_qb[hh * 64:(hh + 1) * 64, pr, :],
                        in0=oaugs[row][0:64, :], in1=rb[:], op=MULT)
            for ns in range(4):
                for co in range(2):
                    pj = ps.tile([128, 512], F32, tag="acc", bufs=4)
                    for pr in range(4):
                        nc.tensor.matmul(pj[:],
                                         O_qb[:, pr, ns * 128:(ns + 1) * 128],
                                         pw_sb[:, pr, co * 512:(co + 1) * 512],
                                         start=(pr == 0), stop=(pr == 3))
                    so = sb.tile([128, 512], F32, tag="stout", bufs=3)
                    if co == 0:
                        nc.vector.tensor_copy(out=so[:], in_=pj[:])
                    else:
                        nc.scalar.copy(out=so[:], in_=pj[:])
                    nc.sync.dma_start(
                        out[q0 + ns * 128:q0 + (ns + 1) * 128,
                            co * 512:(co + 1) * 512], so[:])

        pending = None
        for qb_i in range(4):
            O_qb = sb.tile([128, 4, 512], BF16, tag="oqb", bufs=2)
            den8 = sb.tile([8, 512], F32, tag="den8", bufs=2)
            oaugs = []
            for pr in range(4):
                attn_block(qb_i, pr, oaugs, den8)
                if pr == 0 and pending is not None:
                    boundary_chunk(*pending)
            pending = (qb_i, O_qb, den8, oaugs)
        boundary_chunk(*pending)
    return nc


def _to_bf16(a: np.ndarray) -> np.ndarray:
    return np.ascontiguousarray(a.astype(ml_dtypes.bfloat16))


def _prepare_in_maps(x, qkv_w, qkv_b, proj_w):
    xr = _round_fp32r(x)
    wr = _round_fp32r(qkv_w)
    qkv_b = np.asarray(qkv_b, dtype=np.float32)
    in_maps = []
    for c in range(8):
        b, g = c % 4, c // 4
        w0 = 512 * g
        in_maps.append({
            "xT": np.ascontiguousarray(xr[b].T),
            "wcat": np.ascontiguousarray(np.concatenate(
                [wr[:, w0:w0 + 512],
                 wr[:, 1024 + w0:1024 + w0 + 512],
                 wr[:, 2048 + w0:2048 + w0 + 512]], axis=1)),
            "qb": np.ascontiguousarray(qkv_b[w0:w0 + 512].reshape(4, 128).T),
            "kb": np.ascontiguousarray(
                qkv_b[1024 + w0:1024 + w0 + 512].reshape(4, 128).T),
            "pw": _to_bf16(proj_w[w0:w0 + 512, :]),
        })
    return in_maps


def _gather(parts, qkv_b, proj_w, proj_b):
    const_row = (np.asarray(qkv_b)[2048:].astype(np.float64)
                 @ np.asarray(proj_w).astype(np.float64)
                 + np.asarray(proj_b).astype(np.float64))
    out = np.empty((B, N, C), np.float32)
    for b in range(B):
        out[b] = (parts[b].astype(np.float64) + parts[b + 4].astype(np.float64)
                  + const_row).astype(np.float32)
    return out


def kernel(**inputs: np.ndarray) -> np.ndarray:
    x = np.asarray(inputs["x"], dtype=np.float32)
    qkv_w = np.asarray(inputs["qkv_w"], dtype=np.float32)
    qkv_b = np.asarray(inputs["qkv_b"], dtype=np.float32)
    proj_w = np.asarray(inputs["proj_w"], dtype=np.float32)
    proj_b = np.asarray(inputs["proj_b"], dtype=np.float32)

    in_maps = _prepare_in_maps(x, qkv_w, qkv_b, proj_w)
    nc = _build()
    nc.finalize()
    res = run_bass_kernel_spmd(nc, in_maps, list(range(8)))
    parts = [res.results[c]["out"] for c in range(8)]
    return _gather(parts, qkv_b, proj_w, proj_b)


if __name__ == "__main__":
    import tempfile
    import time

    from concourse.bass_utils import compile_bass_kernel

    t0 = time.time()
    nc = _build()
    nc.compile()
    with tempfile.TemporaryDirectory() as td:
        compile_bass_kernel(nc, td, neff_name="k.neff")
    print(f"COMPILE OK ({time.time() - t0:.0f}s)", flush=True)
